# revision 1
# baseline (speedup 1.0000x reference)
"""Trainium2 Bass kernel (raw Bass, no Tile): per-class precision/recall sums.

Computes, for pred/gt 0-1 indicator tensors of shape [N, C]:
    intersection = sum_n pred*gt   [C]
    pred_sum     = sum_n pred      [C]
    gt_sum       = sum_n gt        [C]
    precisions   = (intersection + EPS) / (pred_sum + EPS)
    recalls      = (intersection + EPS) / (gt_sum + EPS)

Sharding: rows split across 8 NeuronCores. The host interleaves each
core's pred/gt chunks in 8-row blocks into x[R/8, 2, 8, C] so one DMA
per tile feeds both operands (each 128-element run purely pred or gt).
Each core emits a [1, 3*C] partial; the host sums partials (integer
values < 2^24, exact in fp32) and applies the epsilon math.

Device pipeline per core (memory-bound, 64 MiB HBM traffic):
  - gpsimd SWDGE DMAs cast f32 -> bf16 in flight (exact for 0/1):
    32 tiles xt[128, 4096] bf16 into 16 rotating SBUF slots.
  - TensorE does all the math:
    * ones[128,1]^T @ 512-col slices accumulate ps_sums[1,512].
    * Gram matmuls pred_run^T @ gt_run per 256-col block accumulate
      ps_gram[128,128]; diagonal entry a=(r,c) = pred.gt dot.
  - Epilogue: diag-mask ps_gram (affine_select identity), fp32
    ones-matmul column-sum -> ps_row[1,128], strided reduces fold into
    res[1,48] = [pred_sum, gt_sum, intersection].

Raw Bass because this compiler build encodes at most ONE semaphore wait
per TPB instruction: all multi-condition waits are standalone sequencer
wait_ge instructions. Correctness argument for slot recycling: the DMA
for tile t (t>=16) waits pe_sem >= t-15, i.e. PE finished reading tile
t-16 in that slot, which also implies that DMA t-16 completed.
Per-slot DMA-completion sems make PE's data waits exact even if the
runtime spreads DMAs across queues that complete out of order.
"""

from contextlib import ExitStack

import numpy as np

N_CORES = 8
N_ROWS, C = 4194304, 16
ROWS_PER_CORE = N_ROWS // N_CORES  # 524288
EPS = np.float32(1e-6)

P = 128
ELEMS_PER_CORE = ROWS_PER_CORE * 2 * C      # 16777216
FREE = 8192          # bf16 elements per partition per tile
TILE_ELEMS = P * FREE                       # 1048576
N_TILES = ELEMS_PER_CORE // TILE_ELEMS      # 16
N_SLOTS = 10
MM_FREE = 512
N_SUM_SLICES = FREE // MM_FREE              # 16
GRAM_BLK = 256       # (two=2, r=8, c=16)
N_GRAM_BLKS = FREE // GRAM_BLK              # 32

_CACHE = {}
LAST_RUN = None  # BassKernelResults of the most recent run (for test harness)


def _build_nc():
    import concourse.bass as bass
    import concourse.mybir as mybir

    f32 = mybir.dt.float32
    bf16 = mybir.dt.bfloat16

    nc = bass.Bass()
    x_d = nc.dram_tensor("x", [ROWS_PER_CORE // 8, 2, 8, C], f32,
                         kind="ExternalInput")
    out_d = nc.dram_tensor("out", [1, 3 * C], f32, kind="ExternalOutput")

    x_t = x_d[:, :, :, :].rearrange("(t p f) two r c -> t p (f two r c)",
                                    p=P, f=FREE // GRAM_BLK)

    ctx = ExitStack()
    with ctx:
        ones_b = ctx.enter_context(nc.sbuf_tensor("ones_b", [P, 1], bf16))
        ones_f = ctx.enter_context(nc.sbuf_tensor("ones_f", [P, 1], f32))
        onesI = ctx.enter_context(nc.sbuf_tensor("onesI", [P, P], f32))
        ident = ctx.enter_context(nc.sbuf_tensor("ident", [P, P], f32))
        diag = ctx.enter_context(nc.sbuf_tensor("diag", [P, P], f32))
        sum4 = ctx.enter_context(nc.sbuf_tensor("sum4", [1, 4 * C], f32))
        res = ctx.enter_context(nc.sbuf_tensor("res", [1, 3 * C], f32))
        slots = [
            ctx.enter_context(nc.sbuf_tensor(f"xt{s}", [P, FREE], bf16))
            for s in range(N_SLOTS)
        ]

        ps_sums = ctx.enter_context(nc.psum_tensor([1, MM_FREE], f32))
        ps_gram = ctx.enter_context(nc.psum_tensor([P, P], f32))
        ps_row = ctx.enter_context(nc.psum_tensor([1, P], f32))

        slot_sems = [
            ctx.enter_context(nc.semaphore(name=f"slot{s}"))
            for s in range(N_SLOTS)
        ]
        qsems = [
            ctx.enter_context(nc.semaphore(name=f"q{k}"))
            for k in range(4)
        ]
        pe_sem = ctx.enter_context(nc.semaphore(name="pe"))
        dve_sem = ctx.enter_context(nc.semaphore(name="dve"))
        pool_sem = ctx.enter_context(nc.semaphore(name="pool"))
        out_sem = ctx.enter_context(nc.semaphore(name="outd"))
        block = ctx.enter_context(nc.Block())

        @block.gpsimd
        def _(gpsimd):
            gpsimd.memset(onesI[:], 1.0)
            gpsimd.affine_select(ident[:], onesI[:], [[1, P]],
                                 mybir.AluOpType.is_equal, 0.0,
                                 base=0, channel_multiplier=-1)
            gpsimd.nop().then_inc(pool_sem, 1)
            for t in range(N_TILES):
                s = t % N_SLOTS
                if t >= N_SLOTS:
                    # PE finished reading the previous occupant of this slot
                    gpsimd.wait_ge(pe_sem, t - N_SLOTS + 1)
                if t < N_TILES - 1:
                    gpsimd.dma_start(slots[s][:], x_t[t]).then_inc(
                        slot_sems[s], 16)
                else:
                    # last tile: 4 quarter-DMAs so PE can chase the stream
                    # and finish right after the final byte lands
                    q = FREE // 4
                    for k in range(4):
                        gpsimd.dma_start(
                            slots[s][:, k * q:(k + 1) * q],
                            x_t[t][:, k * q:(k + 1) * q],
                        ).then_inc(qsems[k], 16)
            # final output DMA after DVE finishes the epilogue
            gpsimd.wait_ge(dve_sem, 3)
            gpsimd.dma_start(out_d[:, :], res[:]).then_inc(out_sem, 16)
            gpsimd.wait_ge(out_sem, 16)

        @block.vector
        def _(vector):
            vector.memset(ones_b[:], 1.0)
            vector.memset(ones_f[:], 1.0)
            vector.nop().then_inc(dve_sem, 1)
            # epilogue part 1: after all accumulation matmuls
            vector.wait_ge(pe_sem, N_TILES)
            vector.wait_ge(pool_sem, 1)
            vector.tensor_mul(diag[:], ps_gram[:, :], ident[:])
            vector.tensor_reduce(
                sum4[:],
                ps_sums[:, :].rearrange("p (b2 two r c) -> p b2 two c r",
                                        b2=2, two=2, r=8, c=C),
                axis=mybir.AxisListType.X, op=mybir.AluOpType.add)
            vector.tensor_reduce(
                res[0:1, 0:2 * C],
                sum4[:, :].rearrange("p (b2 tc) -> p tc b2", b2=2, tc=2 * C),
                axis=mybir.AxisListType.X, op=mybir.AluOpType.add)
            vector.nop().then_inc(dve_sem, 1)  # diag + sums folded
            # epilogue part 2: after PE's diag column-sum matmul
            vector.wait_ge(pe_sem, N_TILES + 1)
            vector.tensor_reduce(
                res[0:1, 2 * C:3 * C],
                ps_row[:, :].rearrange("p (g c) -> p c g", g=8, c=C),
                axis=mybir.AxisListType.X, op=mybir.AluOpType.add)
            vector.nop().then_inc(dve_sem, 1)

        @block.tensor
        def _(tensor):
            tensor.wait_ge(dve_sem, 1)  # ones_b / ones_f ready
            for t in range(N_TILES):
                s = t % N_SLOTS
                xt = slots[s]
                quarters = 1 if t < N_TILES - 1 else 4
                if quarters == 1:
                    tensor.wait_ge(slot_sems[s], 16 * (t // N_SLOTS + 1))
                for k in range(quarters):
                    if quarters == 4:
                        tensor.wait_ge(qsems[k], 16)
                    nsum = N_SUM_SLICES // quarters
                    ngram = N_GRAM_BLKS // quarters
                    for i in range(k * nsum, (k + 1) * nsum):
                        mm = t * N_SUM_SLICES + i
                        nc.tensor.matmul(
                            ps_sums[:, :], ones_b[:],
                            xt[:, i * MM_FREE:(i + 1) * MM_FREE],
                            start=(mm == 0),
                            stop=(mm == N_TILES * N_SUM_SLICES - 1))
                    for j in range(k * ngram, (k + 1) * ngram):
                        mm = t * N_GRAM_BLKS + j
                        base = j * GRAM_BLK
                        mminst = nc.tensor.matmul(
                            ps_gram[:, :], xt[:, base:base + P],
                            xt[:, base + P:base + 2 * P],
                            start=(mm == 0),
                            stop=(mm == N_TILES * N_GRAM_BLKS - 1))
                        if j == N_GRAM_BLKS - 1:
                            mminst.then_inc(pe_sem, 1)
            # epilogue: fp32 column-sum of masked diagonal
            tensor.wait_ge(dve_sem, 2)
            nc.tensor.matmul(ps_row[:, :], ones_f[:], diag[:],
                             start=True, stop=True).then_inc(pe_sem, 1)

    return nc


def _get_nc():
    if "nc" not in _CACHE:
        _CACHE["nc"] = _build_nc()
    return _CACHE["nc"]


def kernel(pred, gt, **run_kwargs):
    global LAST_RUN
    from concourse.bass_utils import run_bass_kernel_spmd

    pred = np.asarray(pred, dtype=np.float32)
    gt = np.asarray(gt, dtype=np.float32)
    assert pred.shape == (N_ROWS, C) and gt.shape == (N_ROWS, C)

    in_maps = []
    for i in range(N_CORES):
        sl = slice(i * ROWS_PER_CORE, (i + 1) * ROWS_PER_CORE)
        x = np.empty((ROWS_PER_CORE // 8, 2, 8, C), dtype=np.float32)
        x[:, 0, :, :] = pred[sl].reshape(-1, 8, C)
        x[:, 1, :, :] = gt[sl].reshape(-1, 8, C)
        in_maps.append({"x": x})

    nc = _get_nc()
    br = run_bass_kernel_spmd(nc, in_maps, core_ids=list(range(N_CORES)),
                              **run_kwargs)
    LAST_RUN = br

    partials = np.stack([r["out"].reshape(3 * C) for r in br.results])
    totals = partials.astype(np.float64).sum(axis=0)  # exact integers
    pred_sum = totals[0:C].astype(np.float32)
    gt_sum = totals[C:2 * C].astype(np.float32)
    intersection = totals[2 * C:3 * C].astype(np.float32)

    recalls = (intersection + EPS) / (gt_sum + EPS)
    precisions = (intersection + EPS) / (pred_sum + EPS)
    return (precisions, recalls, intersection, gt_sum, pred_sum)



# revision 2
# speedup vs baseline: 2.6567x; 2.6567x over previous
"""Trainium2 Bass kernel (raw Bass): per-class precision/recall sums.

Computes, for pred/gt 0-1 indicator tensors of shape [N, C]:
    intersection = sum_n pred*gt   [C]
    pred_sum     = sum_n pred      [C]
    gt_sum       = sum_n gt        [C]
    precisions   = (intersection + EPS) / (pred_sum + EPS)
    recalls      = (intersection + EPS) / (gt_sum + EPS)

Sharding: rows split across 8 NeuronCores. The host re-encodes each
core's chunk as fp8(e5m2) -- exact for 0/1 -- in 226-column blocks
    [pred(7 rows x 16 cls) | 1.0 | gt(same 7 rows) | 1.0]
staged as x[128, 592, 226] (rows per partition padded 4096 -> 592*7 with
zeros; zero rows only pollute the ignored ones*ones cell).

Device: one accumulating matmul per block does ALL the math:
    W = block cols 0:128   = [pred 112 | ones | 15 junk cols]
    R = block cols 113:226 = [gt 112 | ones]
    psum[j, n] += sum_k W[k, j] * R[k, n]
  diag j=n<112   -> intersection per (r, c) slot
  col 112, j<112 -> pred sums per slot
  row 112, n<112 -> gt sums per slot
  rows 113-127, cell (112,112): junk, ignored on host.
The 128-wide weight window keeps Fast-Weight-Load enabled (needs
exactly 128 weight cols, non-fp32) so LDWEIGHTS hides under the
113-cycle stream of the neighboring matmul.

The whole 131 KiB/partition payload fits in SBUF, so all input DMAs
are issued back-to-back up front (no slot recycling) and the PE chases
them with per-tile semaphores; the last tile is split into 4 quarter
DMAs so PE finishes right after the final byte lands. Epilogue: DVE
copies psum -> SBUF, DMA out [128, 113] fp32 partials; the host folds
the 7 row-groups and sums cores in float64 (exact integers).
"""

from contextlib import ExitStack

import numpy as np

N_CORES = 8
N_ROWS, C = 4194304, 16
ROWS_PER_CORE = N_ROWS // N_CORES   # 524288
EPS = np.float32(1e-6)

P = 128
RPP = ROWS_PER_CORE // P            # 4096 rows per partition
R_GRP = 7                           # row-groups per block
D = R_GRP * C                       # 112 data cols per tensor per block
BLK_W = 2 * D + 2                   # 226
GT_OFF = D + 1                      # 113
M_OUT = D + 1                       # 113 meaningful out rows/cols
W_COLS = 128                        # weight window (FWL needs 128)
N_BLOCKS = 592                      # 592*7 = 4144 row slots (48 pad)
N_TILES = 16
BPT = N_BLOCKS // N_TILES           # 37 blocks per tile DMA
LAST_Q = (10, 9, 9, 9)              # last tile quarter-DMA block counts

ONE_E5M2 = np.uint8(0x3C)           # bit pattern of 1.0 in fp8 e5m2

_CACHE = {}
LAST_RUN = None  # BassKernelResults of the most recent run (for test harness)


def _build_nc():
    import concourse.bass as bass
    import concourse.mybir as mybir

    f32 = mybir.dt.float32
    f8 = mybir.dt.float8e5

    nc = bass.Bass()
    x_d = nc.dram_tensor("x", [P, N_BLOCKS, BLK_W], f8, kind="ExternalInput")
    out_d = nc.dram_tensor("out", [P, M_OUT], f32, kind="ExternalOutput")

    ctx = ExitStack()
    with ctx:
        data = ctx.enter_context(nc.sbuf_tensor("data", [P, N_BLOCKS, BLK_W], f8))
        res = ctx.enter_context(nc.sbuf_tensor("res", [P, M_OUT], f32))
        ps = ctx.enter_context(nc.psum_tensor([P, M_OUT], f32))

        tsems = [
            ctx.enter_context(nc.semaphore(name=f"t{t}"))
            for t in range(N_TILES - 1)
        ]
        qsems = [
            ctx.enter_context(nc.semaphore(name=f"q{k}"))
            for k in range(len(LAST_Q))
        ]
        pe_sem = ctx.enter_context(nc.semaphore(name="pe"))
        dve_sem = ctx.enter_context(nc.semaphore(name="dve"))
        out_sem = ctx.enter_context(nc.semaphore(name="outd"))
        block = ctx.enter_context(nc.Block())

        @block.gpsimd
        def _(gpsimd):
            for t in range(N_TILES - 1):
                gpsimd.dma_start(
                    data[:, t * BPT:(t + 1) * BPT, :],
                    x_d[:, t * BPT:(t + 1) * BPT, :],
                ).then_inc(tsems[t], 16)
            s = (N_TILES - 1) * BPT
            for k, n in enumerate(LAST_Q):
                gpsimd.dma_start(
                    data[:, s:s + n, :], x_d[:, s:s + n, :]
                ).then_inc(qsems[k], 16)
                s += n
            gpsimd.wait_ge(dve_sem, 1)
            gpsimd.dma_start(out_d[:, :], res[:, :]).then_inc(out_sem, 16)
            gpsimd.wait_ge(out_sem, 16)

        @block.tensor
        def _(tensor):
            def do_block(b):
                return nc.tensor.matmul(
                    ps[:, :],
                    data[:, b, 0:W_COLS],
                    data[:, b, GT_OFF:GT_OFF + M_OUT],
                    start=(b == 0),
                    stop=(b == N_BLOCKS - 1),
                )

            for t in range(N_TILES - 1):
                tensor.wait_ge(tsems[t], 16)
                for b in range(t * BPT, (t + 1) * BPT):
                    do_block(b)
            s = (N_TILES - 1) * BPT
            inst = None
            for k, n in enumerate(LAST_Q):
                tensor.wait_ge(qsems[k], 16)
                for b in range(s, s + n):
                    inst = do_block(b)
                s += n
            inst.then_inc(pe_sem, 1)

        @block.vector
        def _(vector):
            vector.wait_ge(pe_sem, 1)
            vector.tensor_copy(res[:, :], ps[:, :])
            vector.nop().then_inc(dve_sem, 1)

    return nc


def _get_nc():
    if "nc" not in _CACHE:
        _CACHE["nc"] = _build_nc()
    return _CACHE["nc"]


def _stage_core(pred_u8, gt_u8):
    """pred_u8/gt_u8: [ROWS_PER_CORE, C] uint8 0/1 -> x[P, N_BLOCKS, BLK_W]
    fp8e5m2 bit pattern (as uint8)."""
    x = np.empty((P, N_BLOCKS, BLK_W), dtype=np.uint8)
    pad = np.zeros((P, N_BLOCKS * R_GRP - RPP, C), dtype=np.uint8)

    pb = np.concatenate([pred_u8.reshape(P, RPP, C), pad], axis=1)
    x[:, :, 0:D] = pb.reshape(P, N_BLOCKS, D) * ONE_E5M2
    x[:, :, D] = ONE_E5M2

    gb = np.concatenate([gt_u8.reshape(P, RPP, C), pad], axis=1)
    x[:, :, GT_OFF:GT_OFF + D] = gb.reshape(P, N_BLOCKS, D) * ONE_E5M2
    x[:, :, GT_OFF + D] = ONE_E5M2
    return x


def kernel(pred, gt, **run_kwargs):
    global LAST_RUN
    import ml_dtypes
    from concourse.bass_utils import run_bass_kernel_spmd

    pred = np.asarray(pred)
    gt = np.asarray(gt)
    assert pred.shape == (N_ROWS, C) and gt.shape == (N_ROWS, C)

    pred_u8 = pred.astype(np.uint8)   # 0/1
    gt_u8 = gt.astype(np.uint8)

    in_maps = []
    for i in range(N_CORES):
        sl = slice(i * ROWS_PER_CORE, (i + 1) * ROWS_PER_CORE)
        x = _stage_core(pred_u8[sl], gt_u8[sl])
        in_maps.append({"x": x.view(ml_dtypes.float8_e5m2)})

    nc = _get_nc()
    br = run_bass_kernel_spmd(nc, in_maps, core_ids=list(range(N_CORES)),
                              **run_kwargs)
    LAST_RUN = br

    # Sum the [128, 113] per-core partials exactly, then fold the
    # 7 row-groups per class.
    T = np.zeros((P, M_OUT), dtype=np.float64)
    for r in br.results:
        T += np.asarray(r["out"], dtype=np.float64)

    diag = np.diagonal(T)[:D]                       # intersection slots
    intersection = diag.reshape(R_GRP, C).sum(axis=0).astype(np.float32)
    pred_sum = T[:D, D].reshape(R_GRP, C).sum(axis=0).astype(np.float32)
    gt_sum = T[D, :D].reshape(R_GRP, C).sum(axis=0).astype(np.float32)

    recalls = (intersection + EPS) / (gt_sum + EPS)
    precisions = (intersection + EPS) / (pred_sum + EPS)
    return (precisions, recalls, intersection, gt_sum, pred_sum)
